# revision 1
# baseline (speedup 1.0000x reference)
"""Trainium2 Bass kernel for nn_Min_interval (subset min-interval selection).

Problem: for each batch row, for every subset S of the 16 input columns with
|S| <= 3, output the (xl, xu) interval of the column in S minimizing the
score s = 0.5*xl + 0.5*xu (ties broken by b = 0.2*xl + 0.8*xu, then by DP
fold order).  Output columns are sorted by subset bitmask -> [B, 696] x 2.

Device algorithm (per core, pure data-parallel over rows):
  *  s~ = xl + xu compares identically to 0.5*xl+0.5*xu (exact halving), so
     the kernel compares s~ and the winner of any subset is argmin s~.
  *  M2 = the 136 subsets with |S|<=2 in bitmask order have a prefix
     structure: block j = [single j, pair(0,j), .., pair(j-1,j)] and every
     pair block's left operands are the contiguous singles prefix.
  *  The full output in bitmask order is block t = [single t] ++
     [select(M2[k], single t) for k < c2(t)], where c2(t) = t(t+1)/2 is the
     number of |S|<=2 subsets with bitmask < 2^t.  So every output block is
     one uniform select of a *contiguous prefix* of the M2 staging array
     against one broadcast column.
  *  A sentinel column with s~ = +inf is prepended to both the input staging
     and M2 staging arrays; select(sentinel, single t) = single t, which
     makes the leading "single" element of every block fall out of the same
     vectorized op (no scattered scalar copies anywhere).
  *  Each select is: is_gt mask (Vector engine), plain copy of the on-false
     prefix (Scalar engine, merged l/u/s~ planes via 4-dim APs), and
     copy_predicated of the broadcast on-true column (Vector engine).  The
     s~ plane of the select result doubles as min(left,right), feeding the
     next stage.
  *  Outputs stream to HBM in two column waves per chunk, split across the
     SP-HWDGE and SWDGE DMA queues so stores overlap compute and the next
     chunk's input loads.
  *  Exact s~ ties between distinct columns (~1e-7 of rows) are detected on
     host and those rows are recomputed with exact reference semantics in
     numpy, so the device kernel needs strict > only.

Sharding: batch 65536 rows -> 8 cores x 8192 rows (data parallel, no comms).
Measured: ~204 us/core CoreSim estimate, ~200 us/core on HW (per-core HBM
write roofline for the 45.6 MB output slice is ~127 us; the Vector engine is
the bottleneck at ~100% busy — it owns all compares and predicated copies).
"""

import os
import sys
import numpy as np

for _p in ("/opt/trn_rl_repo",):
    if _p not in sys.path and os.path.isdir(_p):
        sys.path.insert(0, _p)

# ----------------------------------------------------------------------------
# Problem constants (hardcoded per contest rules)
# ----------------------------------------------------------------------------
N = 16                 # input feature columns
ADD = 3                # max subset order
ALPHA = 0.5
BETA = 0.8
BATCH = 65536
N_CORES = 8
ROWS_PER_CORE = BATCH // N_CORES        # 8192
P = 128                                 # SBUF partitions
OUT_COLS = 696                          # subsets with 1<=|S|<=3 of 16
NB_DEFAULT = 16                         # rowblocks per chunk

# triangular prefix counts: c2[t] = #subsets |S|<=2 with bitmask < 2^t
C2 = [t * (t + 1) // 2 for t in range(N + 1)]
# output block offsets: block t (top bit t) starts at BOFF[t], width 1+c2[t]
BOFF = [0] * (N + 1)
for _t in range(N):
    BOFF[_t + 1] = BOFF[_t] + 1 + C2[_t]
assert BOFF[N] == OUT_COLS

M2_COLS = C2[N]  # 136 = number of |S|<=2 subsets (120 pairs + 16 singles)

# SBUF staging layouts (all rowblock-major along the free dim)
C_INQ = N + 1            # 17: sentinel + 16 input cols, per plane
C_IN = 3 * C_INQ         # 51: l, u, s~ planes
C_P2Q = 1 + M2_COLS      # 137: sentinel + M2
C_P2 = 3 * C_P2Q         # 411
C_OUT = 2 * OUT_COLS     # 1392


# ----------------------------------------------------------------------------
# Bass program builder
# ----------------------------------------------------------------------------
def build_program(rows=ROWS_PER_CORE, nb=NB_DEFAULT, reps=1):
    """Build the per-core Bass program. rows must be divisible by 128*nb.

    reps repeats the whole computation in-program (benchmarking only).
    """
    from contextlib import ExitStack
    from concourse import bacc, mybir, tile

    f32 = mybir.dt.float32
    u32 = mybir.dt.uint32
    gt = mybir.AluOpType.is_gt

    chunks = rows // (P * nb)
    assert chunks * P * nb == rows

    nc = bacc.Bacc()
    xl_d = nc.declare_dram_parameter("xl", [rows, N], f32, isOutput=False)
    xu_d = nc.declare_dram_parameter("xu", [rows, N], f32, isOutput=False)
    ol_d = nc.declare_dram_parameter("out_l", [rows, OUT_COLS], f32, isOutput=True)
    ou_d = nc.declare_dram_parameter("out_u", [rows, OUT_COLS], f32, isOutput=True)

    # DRAM views: (chunk, partition, rowblock, col)
    xl_r = xl_d[:].rearrange("(c nb p) t -> c p nb t", nb=nb, p=P)
    xu_r = xu_d[:].rearrange("(c nb p) t -> c p nb t", nb=nb, p=P)
    ol_r = ol_d[:].rearrange("(c nb p) o -> c p nb o", nb=nb, p=P)
    ou_r = ou_d[:].rearrange("(c nb p) o -> c p nb o", nb=nb, p=P)

    out_bufs = 2 if nb <= 8 else 1
    with ExitStack() as ctx:
        tc = ctx.enter_context(tile.TileContext(nc))
        inp = ctx.enter_context(tc.tile_pool(name="inp", bufs=2))
        p2p = ctx.enter_context(tc.tile_pool(name="p2p", bufs=2))
        outp = ctx.enter_context(tc.tile_pool(name="outp", bufs=out_bufs))
        outpB = ctx.enter_context(tc.tile_pool(name="outpB", bufs=1))
        mp = ctx.enter_context(tc.tile_pool(name="mp", bufs=3))

        for _rep in range(reps):
          for ch in range(chunks):
            inb = inp.tile([P, nb * C_IN], f32, tag="inb")
            # [p, nb, c] and [p, v, nb, q] views of the input staging tile
            in3 = inb[:].rearrange("p (nb c) -> p nb c", c=C_IN)
            in4 = inb[:].rearrange("p (nb v q) -> p v nb q", v=3, q=C_INQ)

            # load xl -> l plane cols 1..16, xu -> u plane cols 1..16
            nc.sync.dma_start(out=in3[:, :, 1:1 + N], in_=xl_r[ch])
            nc.sync.dma_start(out=in3[:, :, C_INQ + 1:C_INQ + 1 + N], in_=xu_r[ch])

            # s~ plane: sentinel = +inf, cols 1..16 = l + u
            soff = 2 * C_INQ
            nc.gpsimd.memset(in3[:, :, soff:soff + 1], float("inf"))
            nc.vector.tensor_add(
                in3[:, :, soff + 1:soff + 1 + N],
                in3[:, :, 1:1 + N],
                in3[:, :, C_INQ + 1:C_INQ + 1 + N],
            )

            p2 = p2p.tile([P, nb * C_P2], f32, tag="p2")
            p23 = p2[:].rearrange("p (nb c) -> p nb c", c=C_P2)
            p24 = p2[:].rearrange("p (nb v q) -> p v nb q", v=3, q=C_P2Q)
            s2off = 2 * C_P2Q
            nc.gpsimd.memset(p23[:, :, s2off:s2off + 1], float("inf"))

            # ---------------- pairs stage: fill M2 staging ----------------
            # group j writes M2 block j = [single j, pair(0,j)..pair(j-1,j)]
            # at q = 1+c2(j) .. 1+c2(j)+j   (q=0 is the sentinel)
            for j in range(N):
                W = j + 1
                q0 = 1 + C2[j]
                # left operand: sentinel + singles 0..j-1  (s~ cols 0..j)
                ls = in3[:, :, soff:soff + W]
                # broadcast right operand: single j
                rs = in3[:, :, soff + 1 + j:soff + 2 + j].to_broadcast((P, nb, W))

                pm = mp.tile([P, nb * N], u32, tag="pm")
                pm3 = pm[:].rearrange("p (nb w) -> p nb w", w=N)[:, :, :W]
                nc.vector.tensor_tensor(pm3, ls, rs, gt)

                # l,u,s~ of winners in one shot: copy left prefix, overwrite
                # with right where mask (v=3 planes via 4-dim APs).  The s~
                # plane's select result equals min(left,right) since the mask
                # is exactly (left > right).
                dst = p24[:, 0:3, :, q0:q0 + W]
                nc.scalar.copy(dst, in4[:, 0:3, :, 0:W])
                data = in4[:, 0:3, :, 1 + j:2 + j].to_broadcast((P, 3, nb, W))
                maskb = pm3.unsqueeze(1).to_broadcast((P, 3, nb, W))
                nc.vector.copy_predicated(dst, maskb, data)

            # ---------------- final stage: emit output blocks ----------------
            # two wave tensors so wave-A DMAs overlap wave-B compute and the
            # next chunk can start on wave A while wave B drains
            T_SPLIT = 14
            wA = BOFF[T_SPLIT]
            wB = OUT_COLS - wA
            osbA = outp.tile([P, nb * 2 * wA], f32, tag="osbA")
            osbB = outpB.tile([P, nb * 2 * wB], f32, tag="osbB")
            o4A = osbA[:].rearrange("p (nb v c) -> p v nb c", v=2, c=wA)
            o4B = osbB[:].rearrange("p (nb v c) -> p v nb c", v=2, c=wB)

            for t in range(N):
                W = C2[t] + 1
                b0 = BOFF[t]
                ls = p23[:, :, s2off:s2off + W]
                rs = in3[:, :, soff + 1 + t:soff + 2 + t].to_broadcast((P, nb, W))

                fm = mp.tile([P, nb * (C2[N - 1] + 1)], u32, tag="fm")
                fm3 = fm[:].rearrange("p (nb w) -> p nb w", w=C2[N - 1] + 1)[:, :, :W]
                nc.vector.tensor_tensor(fm3, ls, rs, gt)

                if t < T_SPLIT:
                    dst = o4A[:, :, :, b0:b0 + W]
                else:
                    dst = o4B[:, :, :, b0 - wA:b0 - wA + W]
                nc.scalar.copy(dst, p24[:, 0:2, :, 0:W])
                data = in4[:, 0:2, :, 1 + t:2 + t].to_broadcast((P, 2, nb, W))
                maskb = fm3.unsqueeze(1).to_broadcast((P, 2, nb, W))
                nc.vector.copy_predicated(dst, maskb, data)

                if t == T_SPLIT - 1:
                    # wave A out-DMAs overlap wave-B compute
                    nc.sync.dma_start(out=ol_r[ch][:, :, :wA], in_=o4A[:, 0])
                    nc.gpsimd.dma_start(out=ou_r[ch][:, :, :wA], in_=o4A[:, 1])

            nc.sync.dma_start(out=ol_r[ch][:, :, wA:], in_=o4B[:, 0])
            nc.gpsimd.dma_start(out=ou_r[ch][:, :, wA:], in_=o4B[:, 1])

    nc.finalize()
    return nc


# ----------------------------------------------------------------------------
# Exact reference semantics in numpy (for rare s~ tie rows)
# ----------------------------------------------------------------------------
def _build_plan():
    from itertools import combinations

    items = list(range(N))
    index_dict = {(i,): i for i in items}
    count = N
    plan = []
    for length in range(2, min(ADD, N) + 1):
        combos = list(combinations(items, length))
        left = np.array([index_dict[c[1:]] for c in combos], dtype=np.int32)
        right = np.array([index_dict[c[:-1]] for c in combos], dtype=np.int32)
        for c in combos:
            index_dict[c] = count
            count += 1
        plan.append((left, right))

    def bitmask(c):
        m = 0
        for i in c:
            m |= 1 << i
        return m

    order = np.array(
        [index_dict[c] for c in sorted(index_dict, key=bitmask)], dtype=np.int32
    )
    return plan, order


_PLAN_CACHE = None


def _reference_numpy(xl, xu):
    """Bit-exact fp32 reproduction of the jax reference for given rows."""
    global _PLAN_CACHE
    if _PLAN_CACHE is None:
        _PLAN_CACHE = _build_plan()
    plan, order = _PLAN_CACHE
    a0 = np.float32(1.0 - ALPHA)
    a1 = np.float32(ALPHA)
    b0 = np.float32(1.0 - BETA)
    b1 = np.float32(BETA)
    mat_l, mat_u = xl.astype(np.float32), xu.astype(np.float32)
    for left_idx, right_idx in plan:
        ll, lu = mat_l[:, left_idx], mat_u[:, left_idx]
        rl, ru = mat_l[:, right_idx], mat_u[:, right_idx]
        cur = a0 * ll + a1 * lu
        nxt = a0 * rl + a1 * ru
        bcur = b0 * ll + b1 * lu
        bnxt = b0 * rl + b1 * ru
        choose_right = np.where(cur == nxt, bcur > bnxt, cur > nxt)
        res_l = np.where(choose_right, rl, ll)
        res_u = np.where(choose_right, ru, lu)
        mat_l = np.concatenate([mat_l, res_l], axis=1)
        mat_u = np.concatenate([mat_u, res_u], axis=1)
    return mat_l[:, order], mat_u[:, order]


# ----------------------------------------------------------------------------
# Host entry point
# ----------------------------------------------------------------------------
_PROGRAM_CACHE = {}


def _get_program(rows, nb):
    key = (rows, nb)
    if key not in _PROGRAM_CACHE:
        _PROGRAM_CACHE[key] = build_program(rows, nb)
    return _PROGRAM_CACHE[key]


def kernel(xl, xu):
    from concourse.bass_utils import run_bass_kernel_spmd

    xl = np.ascontiguousarray(np.asarray(xl), dtype=np.float32)
    xu = np.ascontiguousarray(np.asarray(xu), dtype=np.float32)
    assert xl.shape == (BATCH, N) and xu.shape == (BATCH, N)

    nc = _get_program(ROWS_PER_CORE, NB_DEFAULT)

    in_maps = []
    for c in range(N_CORES):
        sl = slice(c * ROWS_PER_CORE, (c + 1) * ROWS_PER_CORE)
        in_maps.append({"xl": xl[sl], "xu": xu[sl]})

    res = run_bass_kernel_spmd(nc, in_maps, list(range(N_CORES))).results

    out_l = np.concatenate([r["out_l"] for r in res], axis=0)
    out_u = np.concatenate([r["out_u"] for r in res], axis=0)

    # Patch rows where two distinct columns have exactly equal s~ keys: the
    # device kernel uses strict-> only; the reference tie-breaks via beta
    # score and DP fold order.  (~1e-7 of rows; exact recompute on host.)
    s = xl + xu
    ss = np.sort(s, axis=1)
    bad = (np.diff(ss, axis=1) == 0).any(axis=1)
    rows = np.nonzero(bad)[0]
    if rows.size:
        pl, pu = _reference_numpy(xl[rows], xu[rows])
        out_l[rows] = pl
        out_u[rows] = pu

    return out_l, out_u



# revision 3
# speedup vs baseline: 1.9504x; 1.9504x over previous
"""Trainium2 Bass kernel v2 for nn_Min_interval (subset min-interval selection).

Device computes, for every subset S of the 16 columns with 2 <= |S| <= 3, the
(l, u) interval of the column in S minimizing s~ = xl + xu (exact-f32 keys;
exact ties are patched on host).  Payloads travel as interleaved fp16 (l, u)
pairs viewed as single u32 elements, so each select touches one element per
subset instead of two, and the output stream is half the bytes of f32.

Device column order: [pairs {x,y} sorted by (y,x) : 120] ++
[triples {x,y,t} sorted by (t, y, x) : 560].  Host reinserts the 16 singles
(fp16 of the inputs computed on device would be identical), permutes columns
to subset-bitmask order, casts fp16 -> f32, and patches exact-tie rows with
the reference numpy semantics.

Key structure: pairs region OUT[0:120] in (y,x) order makes the on-false
source of every triples block t a contiguous prefix OUT[0:C(t,2)], and the
pair stage's left operands are contiguous single prefixes LUS[0:j].

Engine split (HW-measured; Pool/GPSIMD tensor ops and Act-queue DMAs are
far slower on real TRN2 than the cost model suggests, so both are avoided):
DVE does masks (is_gt), preds (copy_predicated), pair mins, and the big-t
u32 on-false copies; Act does the fp16 conversions and small/mid-t copies;
all DMA waves ride the SP (sync) HWDGE queue.  Measured ~97us/core vs the
~188us f32 baseline.
"""

import os
import sys
import numpy as np

for _p in ("/opt/trn_rl_repo",):
    if _p not in sys.path and os.path.isdir(_p):
        sys.path.insert(0, _p)

# ----------------------------------------------------------------------------
# Problem constants
# ----------------------------------------------------------------------------
N = 16
ADD = 3
ALPHA = 0.5
BETA = 0.8
BATCH = 65536
N_CORES = 8
ROWS_PER_CORE = BATCH // N_CORES        # 8192
P = 128
NB_DEFAULT = 16

OUT_COLS = 696                          # harness cols (|S| in 1..3)
N_PAIRS = 120
N_TRIPLES = 560
DEV_COLS = N_PAIRS + N_TRIPLES          # 680


def _c2(n):
    return n * (n - 1) // 2


def _c3(n):
    return n * (n - 1) * (n - 2) // 6


# ----------------------------------------------------------------------------
# Bass program builder
# ----------------------------------------------------------------------------
def build_program(rows=ROWS_PER_CORE, nb=NB_DEFAULT, reps=1,
                  cmp_pat="VVVVVVVVVVVVVV", copy_pat="AAAAAAAAAAVVVV",
                  pair_cmp_eng="V", pair_copy_eng="A", add_eng="V",
                  order="asc", out_bufs=None, mask_bufs=3, wave_q=None):
    """Per-core program.  rows must be divisible by 128*nb.

    cmp_pat: mask engine per triples block t=2..15: 'V' = DVE is_gt (1 op),
    'P' = Pool add(-rs) + Pool relu, 'S' = Pool add + Act relu.
    copy_pat: on-false copy engine per block ('A' Act fp16, 'V' DVE u32,
    'P' Pool u32).  pair_*_eng: pair-stage mask/copy engines.
    add_eng: engine for the s~ add.  order: "ilv" interleaves triples block
    t right after pair group t; "asc" emits pairs then triples ascending;
    "desc" emits pairs then triples descending (big blocks first, so big
    output waves fire early and only a small wave tails the chunk).
    """
    from contextlib import ExitStack
    from concourse import bacc, mybir, tile

    f32 = mybir.dt.float32
    fp16 = mybir.dt.float16
    u8 = mybir.dt.uint8
    u32 = mybir.dt.uint32
    gt = mybir.AluOpType.is_gt
    mn = mybir.AluOpType.min
    addop = mybir.AluOpType.add
    RELU = mybir.ActivationFunctionType.Relu

    chunks = rows // (P * nb)
    assert chunks * P * nb == rows

    nc = bacc.Bacc()
    xl_d = nc.declare_dram_parameter("xl", [rows, N], f32, isOutput=False)
    xu_d = nc.declare_dram_parameter("xu", [rows, N], f32, isOutput=False)
    out_d = nc.declare_dram_parameter("out", [rows, DEV_COLS, 2], fp16, isOutput=True)

    # DRAM views: rows = (c p nb) so each partition holds nb consecutive rows
    xl_r = xl_d[:].rearrange("(c p nb) t -> c p (nb t)", p=P, nb=nb)
    xu_r = xu_d[:].rearrange("(c p nb) t -> c p (nb t)", p=P, nb=nb)
    out_r = out_d[:].rearrange("(c p nb) o v -> c p nb (o v)", p=P, nb=nb)

    # output column waves (device cols), each >= 128 LU cols (512B/row desc);
    # boundaries align with triples-block ends so waves fire mid-stage.
    # For "desc" order the pairs+small-t wave completes last and is smallest.
    if order == "desc":
        WAVES = [(484, DEV_COLS), (285, 484), (130, 285), (0, 130)]
    else:
        WAVES = [(0, 140), (140, 285), (285, 484), (484, DEV_COLS)]
    WAVE_Q = ["sync", "sync", "sync", "sync"] if wave_q is None else wave_q

    if out_bufs is None:
        out_bufs = 2 if nb <= 16 else 1
    with ExitStack() as ctx:
        tc = ctx.enter_context(tile.TileContext(nc))
        inp = ctx.enter_context(tc.tile_pool(name="inp", bufs=2))
        keyp = ctx.enter_context(tc.tile_pool(name="keyp", bufs=2))
        outp = ctx.enter_context(tc.tile_pool(name="outp", bufs=out_bufs))
        mp = ctx.enter_context(tc.tile_pool(name="mp", bufs=mask_bufs))

        for _rep in range(reps):
          for ch in range(chunks):
            inl = inp.tile([P, nb * N], f32, tag="inl")
            inu = inp.tile([P, nb * N], f32, tag="inu")
            nc.sync.dma_start(out=inl[:], in_=xl_r[ch])
            nc.sync.dma_start(out=inu[:], in_=xu_r[ch])
            inl3 = inl[:].rearrange("p (nb t) -> p nb t", t=N)
            inu3 = inu[:].rearrange("p (nb t) -> p nb t", t=N)

            # exact f32 keys s~ = xl + xu, plus negated copy for Pool-side
            # mask building (Pool has no is_gt/subtract: mask = relu(ls-rs)
            # computed as max(ls + (-rs), 0); exactly 0 iff ls <= rs)
            skey = keyp.tile([P, nb * N], f32, tag="skey")
            sk = skey[:].rearrange("p (nb t) -> p nb t", t=N)
            nskey = keyp.tile([P, nb * N], f32, tag="nskey")
            nsk = nskey[:].rearrange("p (nb t) -> p nb t", t=N)
            ENG = {"V": nc.vector, "P": nc.gpsimd}
            ENG[add_eng].tensor_add(sk, inl3, inu3)
            need_neg = ("P" in cmp_pat or "S" in cmp_pat
                        or pair_cmp_eng in ("P", "S"))
            if need_neg:
                nc.gpsimd.tensor_scalar_mul(nsk, sk, -1.0)

            # fp16 (l,u) payload pairs of the 16 singles
            lus = keyp.tile([P, nb * N * 2], fp16, tag="lus")
            lus4 = lus[:].rearrange("p (nb q v) -> p nb q v", q=N, v=2)
            nc.scalar.copy(lus4[:, :, :, 0], inl3)
            nc.scalar.copy(lus4[:, :, :, 1], inu3)
            lusu = lus[:].bitcast(u32).rearrange("p (nb q) -> p nb q", q=N)

            # pair keys s2 in (y,x) order
            s2 = keyp.tile([P, nb * N_PAIRS], f32, tag="s2")
            s23 = s2[:].rearrange("p (nb k) -> p nb k", k=N_PAIRS)

            osb = outp.tile([P, nb * DEV_COLS * 2], fp16, tag="osb")
            o4 = osb[:].rearrange("p (nb c v) -> p nb c v", c=DEV_COLS, v=2)
            ou = osb[:].bitcast(u32).rearrange("p (nb c) -> p nb c", c=DEV_COLS)
            ow = osb[:].rearrange("p (nb cv) -> p nb cv", cv=DEV_COLS * 2)

            # ---------------- stages ----------------
            # pair group j: pairs {x,j} x<j at device cols C(j,2)..C(j,2)+j-1
            # triples block t: {x,y,t} y<t at cols 120+C(t,3) .. +C(t,2)-1
            def pair_group(j):
                q0 = _c2(j)
                ls = sk[:, :, 0:j]
                rs = sk[:, :, j:j + 1].to_broadcast((P, nb, j))

                # f32 masks, u32-bitcast for copy_predicated (needs integer
                # mask dtype).  Tile width 16 > max slice width keeps views
                # 3-dim so all pred operands see identical shapes.
                pm = mp.tile([P, nb * N], f32, tag="pm")
                pm3 = pm[:].rearrange("p (nb w) -> p nb w", w=N)[:, :, :j]
                pmu = pm[:].bitcast(u32).rearrange(
                    "p (nb w) -> p nb w", w=N)[:, :, :j]
                if pair_cmp_eng == "V":
                    nc.vector.tensor_tensor(pm3, ls, rs, gt)
                else:
                    nrs = nsk[:, :, j:j + 1].to_broadcast((P, nb, j))
                    nc.gpsimd.tensor_tensor(pm3, ls, nrs, addop)
                    if pair_cmp_eng == "S":
                        nc.scalar.activation(pm3, pm3, RELU)
                    else:
                        nc.gpsimd.tensor_scalar_max(pm3, pm3, 0.0)

                nc.vector.tensor_tensor(s23[:, :, q0:q0 + j], ls, rs, mn)

                # on-false copy: singles 0..j-1; on-true: single j broadcast
                if pair_copy_eng == "A":
                    nc.scalar.copy(o4[:, :, q0:q0 + j, :], lus4[:, :, 0:j, :])
                else:
                    ENG[pair_copy_eng].tensor_copy(ou[:, :, q0:q0 + j],
                                                   lusu[:, :, 0:j])
                data = lusu[:, :, j:j + 1].to_broadcast((P, nb, j))
                nc.vector.copy_predicated(ou[:, :, q0:q0 + j], pmu, data)

            def triple_block(t):
                W = _c2(t)
                b0 = N_PAIRS + _c3(t)
                ls = s23[:, :, 0:W]
                rs = sk[:, :, t:t + 1].to_broadcast((P, nb, W))

                fm = mp.tile([P, nb * 106], f32, tag="fm")
                fm3 = fm[:].rearrange("p (nb w) -> p nb w", w=106)[:, :, :W]
                fmu = fm[:].bitcast(u32).rearrange(
                    "p (nb w) -> p nb w", w=106)[:, :, :W]
                me = cmp_pat[t - 2]
                if me == "V":
                    nc.vector.tensor_tensor(fm3, ls, rs, gt)
                else:
                    nrs = nsk[:, :, t:t + 1].to_broadcast((P, nb, W))
                    nc.gpsimd.tensor_tensor(fm3, ls, nrs, addop)
                    if me == "S":
                        nc.scalar.activation(fm3, fm3, RELU)
                    else:
                        nc.gpsimd.tensor_scalar_max(fm3, fm3, 0.0)

                ce = copy_pat[t - 2]
                if ce == "A":
                    nc.scalar.copy(o4[:, :, b0:b0 + W, :], o4[:, :, 0:W, :])
                else:
                    ENG[ce].tensor_copy(ou[:, :, b0:b0 + W], ou[:, :, 0:W])
                data = lusu[:, :, t:t + 1].to_broadcast((P, nb, W))
                nc.vector.copy_predicated(ou[:, :, b0:b0 + W], fmu, data)
                return b0 + W

            emits = []
            if order == "ilv":
                for j in range(1, N):
                    emits.append(("p", j))
                    if j >= 2:
                        emits.append(("t", j))
            elif order == "desc":
                emits = [("p", j) for j in range(1, N)]
                emits += [("t", t) for t in range(N - 1, 1, -1)]
            else:
                emits = [("p", j) for j in range(1, N)]
                emits += [("t", t) for t in range(2, N)]

            # fire each wave DMA as soon as its columns are all written
            covered = np.zeros(DEV_COLS, dtype=bool)
            fired = [False] * len(WAVES)

            def fire_ready():
                for i, (wa, wb) in enumerate(WAVES):
                    if not fired[i] and covered[wa:wb].all():
                        fired[i] = True
                        eng = {"sync": nc.sync, "scalar": nc.scalar,
                               "gpsimd": nc.gpsimd}[WAVE_Q[i]]
                        eng.dma_start(out=out_r[ch][:, :, 2 * wa:2 * wb],
                                      in_=ow[:, :, 2 * wa:2 * wb])

            for kind_, idx in emits:
                if kind_ == "p":
                    pair_group(idx)
                    covered[_c2(idx):_c2(idx) + idx] = True
                else:
                    triple_block(idx)
                    b0 = N_PAIRS + _c3(idx)
                    covered[b0:b0 + _c2(idx)] = True
                fire_ready()
            assert all(fired) and covered.all()

    nc.finalize()
    return nc


# ----------------------------------------------------------------------------
# Host-side column permutation: harness bitmask order <- device order
# ----------------------------------------------------------------------------
def _host_maps():
    """Returns (harness_cols, kind, src) where for each harness column i:
    kind[i]=0 -> single, src[i]=column index into xl/xu
    kind[i]=1 -> device col, src[i]=device column index."""
    from itertools import combinations

    subsets = []
    for r in (1, 2, 3):
        subsets += list(combinations(range(N), r))
    subsets.sort(key=lambda c: sum(1 << i for i in c))
    kind = np.zeros(len(subsets), dtype=np.int64)
    src = np.zeros(len(subsets), dtype=np.int64)
    for i, c in enumerate(subsets):
        if len(c) == 1:
            kind[i] = 0
            src[i] = c[0]
        elif len(c) == 2:
            x, y = c
            kind[i] = 1
            src[i] = _c2(y) + x
        else:
            x, y, t = c
            kind[i] = 1
            src[i] = N_PAIRS + _c3(t) + _c2(y) + x
    return kind, src


# ----------------------------------------------------------------------------
# Exact reference semantics in numpy (for rare s~ tie rows)
# ----------------------------------------------------------------------------
def _build_plan():
    from itertools import combinations

    items = list(range(N))
    index_dict = {(i,): i for i in items}
    count = N
    plan = []
    for length in range(2, min(ADD, N) + 1):
        combos = list(combinations(items, length))
        left = np.array([index_dict[c[1:]] for c in combos], dtype=np.int32)
        right = np.array([index_dict[c[:-1]] for c in combos], dtype=np.int32)
        for c in combos:
            index_dict[c] = count
            count += 1
        plan.append((left, right))

    def bitmask(c):
        m = 0
        for i in c:
            m |= 1 << i
        return m

    order = np.array(
        [index_dict[c] for c in sorted(index_dict, key=bitmask)], dtype=np.int32
    )
    return plan, order


_PLAN_CACHE = None


def _reference_numpy(xl, xu):
    """Bit-exact fp32 reproduction of the jax reference for given rows."""
    global _PLAN_CACHE
    if _PLAN_CACHE is None:
        _PLAN_CACHE = _build_plan()
    plan, order = _PLAN_CACHE
    a0 = np.float32(1.0 - ALPHA)
    a1 = np.float32(ALPHA)
    b0 = np.float32(1.0 - BETA)
    b1 = np.float32(BETA)
    mat_l, mat_u = xl.astype(np.float32), xu.astype(np.float32)
    for left_idx, right_idx in plan:
        ll, lu = mat_l[:, left_idx], mat_u[:, left_idx]
        rl, ru = mat_l[:, right_idx], mat_u[:, right_idx]
        cur = a0 * ll + a1 * lu
        nxt = a0 * rl + a1 * ru
        bcur = b0 * ll + b1 * lu
        bnxt = b0 * rl + b1 * ru
        choose_right = np.where(cur == nxt, bcur > bnxt, cur > nxt)
        res_l = np.where(choose_right, rl, ll)
        res_u = np.where(choose_right, ru, lu)
        mat_l = np.concatenate([mat_l, res_l], axis=1)
        mat_u = np.concatenate([mat_u, res_u], axis=1)
    return mat_l[:, order], mat_u[:, order]


# ----------------------------------------------------------------------------
# Host entry point
# ----------------------------------------------------------------------------
_PROGRAM_CACHE = {}
_MAPS_CACHE = None


def _get_program(rows, nb):
    key = (rows, nb)
    if key not in _PROGRAM_CACHE:
        _PROGRAM_CACHE[key] = build_program(rows, nb)
    return _PROGRAM_CACHE[key]


def kernel(xl, xu):
    global _MAPS_CACHE
    from concourse.bass_utils import run_bass_kernel_spmd

    xl = np.ascontiguousarray(np.asarray(xl), dtype=np.float32)
    xu = np.ascontiguousarray(np.asarray(xu), dtype=np.float32)
    assert xl.shape == (BATCH, N) and xu.shape == (BATCH, N)

    nc = _get_program(ROWS_PER_CORE, NB_DEFAULT)

    in_maps = []
    for c in range(N_CORES):
        sl = slice(c * ROWS_PER_CORE, (c + 1) * ROWS_PER_CORE)
        in_maps.append({"xl": xl[sl], "xu": xu[sl]})

    res = run_bass_kernel_spmd(nc, in_maps, list(range(N_CORES))).results
    dev = np.concatenate([r["out"] for r in res], axis=0)  # [B, 680, 2] fp16

    if _MAPS_CACHE is None:
        _MAPS_CACHE = _host_maps()
    kind, src = _MAPS_CACHE

    out_l = np.empty((BATCH, OUT_COLS), dtype=np.float32)
    out_u = np.empty((BATCH, OUT_COLS), dtype=np.float32)
    sing = kind == 0
    out_l[:, sing] = xl[:, src[sing]]
    out_u[:, sing] = xu[:, src[sing]]
    dcols = src[~sing]
    out_l[:, ~sing] = dev[:, dcols, 0].astype(np.float32)
    out_u[:, ~sing] = dev[:, dcols, 1].astype(np.float32)

    # rows where two distinct columns tie exactly on s~ = xl + xu need the
    # reference's beta/fold-order tie-break: recompute those rows on host
    s = xl + xu
    ss = np.sort(s, axis=1)
    bad = (np.diff(ss, axis=1) == 0).any(axis=1)
    rows = np.nonzero(bad)[0]
    if rows.size:
        pl, pu = _reference_numpy(xl[rows], xu[rows])
        out_l[rows] = pl
        out_u[rows] = pu

    return out_l, out_u
